# revision 51
# baseline (speedup 1.0000x reference)
"""AAConv2d (attention-augmented conv) Trainium2 kernel, v12.

Data-parallel over batch: 8 images -> 8 NeuronCores, no collectives.
Per core: qkv projection, 8-head attention with relative-position logits
folded into the QK matmul as extra contraction rows, softmax (no max-sub),
PV, out-projection, and a 3x3 conv via 9 shifted matmuls.

v12 (scheduling + PE-cost rework of the v3 baseline; numerics identical):
  - The ACT (scalar) engine runs the 64 softmax exps (~71us, the kernel
    floor) plus only pre-exp-window work (4 proj copies, table-1 stage)
    and tail muls; the exp stream starts ~25-32us instead of ~42us.
  - Loads: x8+wqk merged into one critical fp8 DMA; all non-critical
    bf16 inputs packed host-side into two bulk tensors so they are
    single FIFO units on the sync DMA queue and cannot starve the
    critical loads; head-0/1 rel-table round-trip DMAs on the gpsimd
    queue to dodge sync-queue congestion.
  - Aw rel-table matmuls for heads 3-7 use a w-major copy of q (qw
    tiles, transposed on DVE+GpSimd) so the PE moving operand is
    linear: ~1034ns -> ~390ns per matmul.
  - Per-head blocks are j-paced: the strict in-order PE stream
    interleaves each QK j-pair with PV / conv / proj / table slices
    whose dependencies are guaranteed ready (emitted >= 1 block ahead;
    bigps-ring users placed so their slot dependency is an exp that
    fired during the pacing).  This structure measures ~145-147us with
    low run-to-run variance (the contiguous-QK variant swings
    142-175us run to run).
  - PV uses N=512 DoubleRow matmuls (8/head); conv is front-loaded into
    the pre-exp window and block slots; conv finish copies on DVE.
  - Norm: dent row copy -> reciprocal_approx_fast -> gpsimd partition
    broadcast -> DVE multiply -> packed attP DMA.
Attention-path fp8 error ~8-9% of the attention section, <0.1% of the
global output scale; conv stays bf16.  End-to-end rel err ~0.0023.
"""
import numpy as np
import ml_dtypes

import concourse.bass as bass
import concourse.tile as tile
from concourse import bacc, mybir
from concourse.bass_utils import run_bass_kernel_spmd

F32 = mybir.dt.float32
BF16 = mybir.dt.bfloat16
F8E4 = mybir.dt.float8e4
F8E5 = mybir.dt.float8e5
AF = mybir.ActivationFunctionType
ALU = mybir.AluOpType
DR = mybir.MatmulPerfMode.DoubleRow

B, CIN, H, W = 8, 256, 32, 32
L = H * W
DK, DV, NH = 512, 256, 8
DKH, DVH = DK // NH, DV // NH

QW_HEADS = range(3, 8)   # heads whose Aw matmul uses the w-major q copy

TRACE = False
TRACE_KW = {}
LAST_RESULT = None


def _bf(a):
    return np.ascontiguousarray(a).astype(ml_dtypes.bfloat16)


def _f8(a):
    return np.ascontiguousarray(a).astype(ml_dtypes.float8_e4m3)


def build():
    nc = bacc.Bacc("TRN2", target_bir_lowering=False, debug=False, num_devices=8)

    bulka = nc.dram_tensor("bulka", [128, 6920], BF16, kind="ExternalInput")
    bulkb = nc.dram_tensor("bulkb", [128, 2560], BF16, kind="ExternalInput")
    xw8 = nc.dram_tensor("xw8", [128, 4096], F8E4, kind="ExternalInput")
    woutt = nc.dram_tensor("woutt", [128, 512], F8E4, kind="ExternalInput")
    relz = nc.dram_tensor("relz", [256, 126], BF16, kind="ExternalInput")
    econst = nc.dram_tensor("econst", [128, 1024], BF16, kind="ExternalInput")
    out_d = nc.dram_tensor("out", [512, 1024], F32, kind="ExternalOutput")
    tdram = nc.dram_tensor("tdram", [8, 128, 1024], BF16)  # rel-table scratch

    with tile.TileContext(nc) as tc:
        with (
            tc.tile_pool(name="const", bufs=1) as cpool,
            tc.tile_pool(name="qp", bufs=1) as qpool,
            tc.tile_pool(name="qw", bufs=1) as qwpool,
            tc.tile_pool(name="kp", bufs=1) as kpool,
            tc.tile_pool(name="vp", bufs=1) as vpool,
            tc.tile_pool(name="expp", bufs=10) as epool,
            tc.tile_pool(name="attn", bufs=3) as apool,
            tc.tile_pool(name="attp", bufs=1) as appool,
            tc.tile_pool(name="stage", bufs=2) as stpool,
            tc.tile_pool(name="scratch", bufs=2) as scpool,
            tc.tile_pool(name="small_sb", bufs=4) as sspool,
            tc.tile_pool(name="outsb", bufs=2) as opool,
            tc.tile_pool(name="bigps", bufs=2, space="PSUM") as bigps,
            tc.tile_pool(name="convps", bufs=1, space="PSUM") as cvps,
            tc.tile_pool(name="avps", bufs=2, space="PSUM") as avps,
        ):
            # ---- tiles ----
            xw_sb = cpool.tile([128, 4096], F8E4, tag="xw8")
            x8_sb = xw_sb[:, 0:2048]
            wqk_sb = xw_sb[:, 2048:4096]
            rel_sb = [cpool.tile([128, 126], BF16, tag=f"relz{p}", name=f"relz{p}") for p in range(2)]
            # all non-critical bf16 inputs ride in two packed bulk tensors
            # (single FIFO units on the sync DMA queue, so they cannot
            # starve the critical loads no matter how the scheduler hoists
            # their issue instructions)
            bulka_sb = cpool.tile([128, 6920], BF16, tag="bulka")
            bulkb_sb = cpool.tile([128, 2560], BF16, tag="bulkb")
            xp_sb = [bulka_sb[:, 3460 * c:3460 * c + 1156] for c in range(2)]
            wcv_sb = [bulka_sb[:, 3460 * c + 1156:3460 * c + 3460] for c in range(2)]
            xn_sb = [bulkb_sb[:, 1280 * c:1280 * c + 1024] for c in range(2)]
            wv_sb = [bulkb_sb[:, 1280 * c + 1024:1280 * c + 1280] for c in range(2)]
            wo_sb = cpool.tile([128, 512], F8E4, tag="wo")

            qp = [qpool.tile([128, 1024], BF16, name=f"qp{i}") for i in range(8)]
            qw = {n: qwpool.tile([128, 1024], BF16, name=f"qw{n}") for n in QW_HEADS}
            kp = [kpool.tile([128, 1024], BF16, name=f"kp{i}") for i in range(8)]

            # ---- ALL loads on the sync queue: per-queue transfers are
            # ---- FIFO, so priority order here IS bandwidth order.
            nc.sync.dma_start(xw_sb[:], xw8.ap())
            for p in range(2):
                nc.sync.dma_start(rel_sb[p][:], relz.ap()[128 * p:128 * p + 128, :])

            def sel_load(n, eng):
                aw_b = 64 if n % 2 == 0 else 0
                eng.dma_start(kp[n][aw_b:aw_b + 64, :],
                              econst.ap()[aw_b:aw_b + 64, :])
            sel_load(0, nc.sync)
            sel_load(1, nc.sync)
            nc.sync.dma_start(bulka_sb[:], bulka.ap())
            nc.sync.dma_start(bulkb_sb[:], bulkb.ap())
            nc.sync.dma_start(wo_sb[:], woutt.ap())
            for n in range(2, 8):
                sel_load(n, nc.sync)

            def memzero_q(n, eng="gp"):
                aw_b = 64 if n % 2 == 0 else 0
                if eng == "dve":
                    nc.vector.memset(qp[n][aw_b:aw_b + 64, :], 0.0)
                else:
                    nc.gpsimd.memset(qp[n][aw_b:aw_b + 64, :], 0.0)

            def memzero_qw(n):
                # zero the non-dkh rows of the w-major q copy
                aw_b = 64 if n % 2 == 0 else 0
                nc.gpsimd.memset(qw[n][aw_b:aw_b + 64, :], 0.0)

            def qw_copy(n):
                """w-major copy of qp[n]'s dkh rows: (a b) -> (b a)."""
                base = 0 if n % 2 == 0 else 64
                dst3 = qw[n][base:base + 64, :].rearrange("p (b a) -> p b a", b=32)
                src3 = (qp[n][base:base + 64, :]
                        .rearrange("p (a b) -> p a b", a=32).transpose([0, 2, 1]))
                nc.vector.tensor_copy(dst3[:, 0:16, :], src3[:, 0:16, :])
                nc.gpsimd.tensor_copy(dst3[:, 16:32, :], src3[:, 16:32, :])

            def proj_chunk(m, dest, eng0, eng1):
                """fp8 DoubleRow projection: K=256 in one pass.
                eng0/eng1 ('dve'|'act') do the two PSUM->SBUF copies."""
                ps = bigps.tile([128, 1024], F32, tag="big", name="projps")
                for s in range(2):
                    nc.tensor.matmul(
                        ps[:, 512 * s:512 * s + 512],
                        wqk_sb[:].rearrange("p (k o) -> p k o", k=2)
                        [:, :, 128 * m:128 * m + 128],
                        x8_sb[:].rearrange("p (k s) -> p k s", k=2)
                        [:, :, 512 * s:512 * s + 512],
                        start=True, stop=True,
                        perf_mode=DR,
                    )
                h0 = 2 * (m % 4)
                for eng, dst, src in (
                    (eng0, dest[h0][0:64, :], ps[0:64, :]),
                    (eng1, dest[h0 + 1][64:128, :], ps[64:128, :]),
                ):
                    if eng == "act":
                        nc.scalar.copy(dst, src)
                    else:
                        nc.vector.tensor_copy(dst, src)

            def head_tables(n, stage_eng, transp="split", dq=None):
                dq = dq or nc.sync
                """rel-table matmuls -> stage -> DRAM -> gathers."""
                par = n % 2
                base = 0 if par == 0 else 64
                aw_b = 64 - base
                ah_b = aw_b + 32
                qfull = qp[n][:]
                tps = bigps.tile([128, 1024], F32, tag="big", name="tps")
                if n in qw:
                    # linear moving from the w-major copy
                    for c in range(2):
                        nc.tensor.matmul(
                            tps[0:63, 512 * c:512 * c + 512],
                            rel_sb[par][:, 0:63],
                            qw[n][:, 512 * c:512 * c + 512],
                            start=True, stop=True,
                        )
                else:
                    qsig = (qfull.rearrange("p (a b) -> p a b", a=32)
                            .transpose([0, 2, 1]))
                    for c in range(2):
                        nc.tensor.matmul(
                            tps[0:63, 512 * c:512 * c + 512],
                            rel_sb[par][:, 0:63],
                            qsig[:, 16 * c:16 * c + 16, :],
                            start=True, stop=True,
                        )
                for c in range(2):
                    nc.tensor.matmul(
                        tps[64:127, 512 * c:512 * c + 512],
                        rel_sb[par][:, 63:126],
                        qfull[:, 512 * c:512 * c + 512],
                        start=True, stop=True,
                    )
                stg = stpool.tile([128, 1024], BF16, name="stg")
                if stage_eng == "act":
                    nc.scalar.copy(stg[:], tps[:])
                else:
                    nc.vector.tensor_copy(stg[:], tps[:])
                dq.dma_start(tdram.ap()[n, :, :], stg[:])
                dq.dma_start(
                    qp[n][aw_b:aw_b + 32, :].rearrange("p (a b) -> p a b", a=32),
                    bass.AP(tdram, n * 131072 + 31 * 1024,
                            [[1024, 32], [-992, 32], [1, 32]]),
                )
                sc = scpool.tile([128, 1024], BF16, name="scr")
                dq.dma_start(
                    sc[ah_b:ah_b + 32, :].rearrange("p (a b) -> p a b", a=32),
                    bass.AP(tdram, n * 131072 + 65536 + 31 * 1024,
                            [[1024, 32], [-992, 32], [1, 32]]),
                )
                dst3 = qp[n][ah_b:ah_b + 32, :].rearrange("p (a b) -> p a b", a=32)
                src3 = (sc[ah_b:ah_b + 32, :].rearrange("p (a b) -> p a b", a=32)
                        .transpose([0, 2, 1]))
                nc.vector.tensor_copy(dst3[:, 0:16, :], src3[:, 0:16, :])
                if transp == "dve":
                    nc.vector.tensor_copy(dst3[:, 16:32, :], src3[:, 16:32, :])
                else:
                    nc.gpsimd.tensor_copy(dst3[:, 16:32, :], src3[:, 16:32, :])

            # ---- conv: c-major schedule so c=0 needs only first bulk loads
            conv_sched = [(o, c, tap, hh) for o in range(2) for c in range(2)
                          for tap in range(9) for hh in range(2)]
            conv_ps = {}

            def conv_emit(lo, hi):
                for idx in range(lo, hi):
                    o, c, tap, hh = conv_sched[idx]
                    if (c, tap, hh) == (0, 0, 0):
                        conv_ps[o] = cvps.tile([128, 1024], F32, tag="cv",
                                               name=f"cps{o}")
                    dy, dx = tap // 3, tap % 3
                    rhs = (xp_sb[c][:]
                           .rearrange("p (h w) -> p h w", h=34)
                           [:, dy + 16 * hh:dy + 16 * hh + 16, dx:dx + 32])
                    nc.tensor.matmul(
                        conv_ps[o][:, 512 * hh:512 * hh + 512],
                        wcv_sb[c][:, 256 * tap + 128 * o:256 * tap + 128 * o + 128],
                        rhs,
                        start=(c == 0 and tap == 0),
                        stop=(c == 1 and tap == 8),
                        skip_group_check=True,
                    )

            def conv_finish(o, eng="dve"):
                osb = opool.tile([128, 1024], F32, name=f"osb{o}")
                if eng == "act":
                    nc.scalar.copy(osb[:], conv_ps[o][:])
                else:
                    nc.vector.tensor_copy(osb[:], conv_ps[o][:])
                nc.sync.dma_start(out_d.ap()[128 * o:128 * o + 128, :], osb[:])

            # ---- v projection -> fp8e4 paired stationaries (+1/64 col) ----
            vpair = [vpool.tile([128, 544], F8E4, name=f"vp{m}") for m in range(4)]

            def v_chunk(j):
                # avps slots (shared with PV) so these don't stall the lt
                # rotation in bigps; all 8 run before the first PV use
                ps = avps.tile([128, 256], F32, tag="av", name="vps")
                for c in range(2):
                    nc.tensor.matmul(
                        ps[:], xn_sb[c][:, 128 * j:128 * j + 128], wv_sb[c][:],
                        start=(c == 0), stop=(c == 1),
                    )
                dst = (vpair[j // 2][:, 272 * (j % 2):272 * (j % 2) + 272]
                       .rearrange("p (n c) -> p n c", n=8)[:, :, 0:32])
                nc.vector.tensor_copy(
                    dst, ps[:].rearrange("p (n c) -> p n c", n=8))

            # ---- attention ----
            att_sb = {}
            attP = appool.tile([128, 2048], F8E4, name="attP")
            aps_t = {}

            def qk_head(n, jlo, jhi, ep):
                for j in range(jlo, jhi):
                    lt = bigps.tile([128, 1024], F32, tag="big", name="lt")
                    for c in range(2):
                        nc.tensor.matmul(
                            lt[:, 512 * c:512 * c + 512],
                            kp[n][:, 128 * j:128 * j + 128],
                            qp[n][:, 512 * c:512 * c + 512],
                            start=True, stop=True,
                        )
                    nc.scalar.activation(
                        ep[j // 2][:, 1024 * (j % 2):1024 * (j % 2) + 1024],
                        lt[:], AF.Exp, scale=2.0 ** -12)

            def pv_c(n, ep, c):
                aps_t[(n, c)] = avps.tile([34, 512], F32, tag="av",
                                          name=f"aps{c}")
                for m in range(4):
                    nc.tensor.matmul(
                        aps_t[(n, c)][:],
                        vpair[m][:].rearrange("p (k c) -> p k c", k=2)
                        [:, :, 34 * n:34 * n + 34],
                        ep[m][:].rearrange("p (k s) -> p k s", k=2)
                        [:, :, 512 * c:512 * c + 512],
                        start=(m == 0), stop=(m == 3),
                        perf_mode=DR,
                        skip_group_check=True,
                    )

            # attP row map: heads 4 and 7 swapped (wo rows permuted on the
            # host to match) so head 7 -- the tail-critical one -- sits at
            # partition base 0 and its normalize multiply can write attP
            # directly, skipping the att tile + DMA hop after the last exp.
            ATT_ROW = {0: 0, 1: 32, 2: 64, 3: 96, 4: 96, 5: 32, 6: 64, 7: 0}

            def norm_head(n):
                direct = (n == 7)
                if not direct:
                    att = apool.tile([32, 1024], F8E4, tag="att",
                                     name=f"att{n}")
                    att_sb[n] = att
                for c in range(2):
                    aps = aps_t[(n, c)]
                    dent = sspool.tile([1, 512], F32, tag="dent", name="dent",
                                       bufs=4)
                    nc.vector.tensor_copy(dent[:], aps[32:33, :])
                    recf = sspool.tile([1, 512], F32, tag="recf", name="recf",
                                       bufs=4)
                    nc.vector.reciprocal_approx_fast(out=recf[:], in_=dent[:])
                    bcs = sspool.tile([32, 512], F32, tag="bcs", name="bcs",
                                      bufs=4)
                    nc.gpsimd.partition_broadcast(bcs[:], recf[:])
                    if direct:
                        dst = attP[0:32, 1024 * (n // 4) + 512 * c:
                                   1024 * (n // 4) + 512 * c + 512]
                    else:
                        dst = att[:, 512 * c:512 * c + 512]
                    nc.vector.tensor_tensor(dst, aps[0:32, :], bcs[:],
                                            op=ALU.mult)
                if not direct:
                    nc.sync.dma_start(
                        attP[ATT_ROW[n]:ATT_ROW[n] + 32,
                             1024 * (n // 4):1024 * (n // 4) + 1024],
                        att[:])

            # ---- phase 1: head-0/1 critical path + front-loaded filler.
            # PE sits idle until the head-0 table round trip completes
            # (~30us), so proj, tables 0-3, all of conv o=0, and the whole
            # v-projection are emitted here to fill it.
            memzero_q(0, "dve")
            proj_chunk(0, qp, "dve", "act")     # qp0 (DVE), qp1 (ACT)
            head_tables(0, "dve", transp="dve", dq=nc.gpsimd)
            proj_chunk(4, kp, "dve", "act")     # kp0 (DVE), kp1 (ACT)
            memzero_q(1, "gp")
            proj_chunk(1, qp, "act", "dve")     # qp2 (ACT), qp3 (DVE)
            proj_chunk(5, kp, "act", "dve")     # kp2 (ACT), kp3 (DVE)
            head_tables(1, "act", transp="dve", dq=nc.gpsimd)
            for mm in range(4):
                nc.gpsimd.memset(vpair[mm][:], 1.0 / 64.0)
            memzero_q(2)
            memzero_q(3)
            memzero_qw(3)
            qw_copy(3)
            conv_emit(0, 16)
            for j in range(8):
                v_chunk(j)

            # ---- per-head blocks, j-paced.  The bigps ring (2 slots)
            # couples allocation i to the readers of allocation i-2, so
            # block-internal placement matters:
            #  * proj ps at the qk1 slot: its ring-dep is exp(n,j0);
            #    and lt(n,j3)'s ring-dep becomes the proj copies -- DVE
            #    runs them first-thing in the block, inside the pacing.
            #  * tables tps at the qk3 slot: ring-dep exp(n,j2); lt(n,j5)
            #    then depends on the stage copy (DVE, second in block).
            #  * PV at qk5/qk6 (ep ready long before); conv (own pool)
            #    fills qk4 + block end; norm chains emitted last (latency
            #    tolerant).
            conv_b = {0: [(16, 21), (21, 26)],
                      1: [(26, 31), (31, 36)],
                      3: [(36, 41)], 4: [(41, 46)], 5: [(46, 51)],
                      6: [(51, 56), (56, 60)], 7: [(60, 65)]}
            proj_b = {0: (2, qp), 1: (6, kp), 2: (3, qp), 3: (7, kp)}
            side_b = {0: lambda: (memzero_q(4), memzero_q(5),
                                  memzero_qw(4), qw_copy(4)),
                      1: lambda: (memzero_qw(5), qw_copy(5)),
                      2: lambda: (memzero_q(6), memzero_q(7),
                                  memzero_qw(6), qw_copy(6), conv_finish(0)),
                      3: lambda: (memzero_qw(7), qw_copy(7))}
            tabl_b = {0: 2, 1: 3, 2: 4, 3: 5, 4: 6, 5: 7}

            ep_tiles = {}
            for n in range(8):
                ep = [epool.tile([128, 2048], F8E5, tag="ep", name=f"ep{n}_{m}")
                      for m in range(4)]
                ep_tiles[n] = ep
                cv = conv_b.get(n, [])
                qk_head(n, 0, 2, ep)
                if n in proj_b:
                    m, dest = proj_b[n]
                    proj_chunk(m, dest, "dve", "dve")
                qk_head(n, 2, 4, ep)
                if n in tabl_b:
                    head_tables(tabl_b[n], "dve")
                qk_head(n, 4, 5, ep)
                if cv:
                    conv_emit(*cv[0])
                qk_head(n, 5, 6, ep)
                if n >= 1:
                    pv_c(n - 1, ep_tiles[n - 1], 0)
                qk_head(n, 6, 7, ep)
                if n >= 1:
                    pv_c(n - 1, ep_tiles[n - 1], 1)
                qk_head(n, 7, 8, ep)
                for c_lo, c_hi in cv[1:]:
                    conv_emit(c_lo, c_hi)
                if n in side_b:
                    side_b[n]()
                if n >= 1:
                    norm_head(n - 1)
                    del ep_tiles[n - 1]
            pv_c(7, ep_tiles[7], 0)
            pv_c(7, ep_tiles[7], 1)
            norm_head(7)
            conv_emit(65, 72)
            conv_finish(1)

            # ---- attn out-projection (DoubleRow over packed attn) ----
            for o in range(2):
                ps = bigps.tile([128, 1024], F32, tag="big", name="pout")
                for c in range(2):
                    nc.tensor.matmul(
                        ps[:, 512 * c:512 * c + 512],
                        wo_sb[:].rearrange("p (k o) -> p k o", k=2)
                        [:, :, 128 * o:128 * o + 128],
                        attP[:].rearrange("p (k s) -> p k s", k=2)
                        [:, :, 512 * c:512 * c + 512],
                        start=True, stop=True,
                        perf_mode=DR,
                    )
                osb = opool.tile([128, 1024], F32, name="osb")
                nc.scalar.mul(osb[:], ps[:], 1.0 / 4096.0)
                nc.sync.dma_start(out_d.ap()[256 + 128 * o:384 + 128 * o, :],
                                  osb[:])

    nc.compile()
    return nc


_NC_CACHE = None


def kernel(x, w_qkv, w_conv, w_out, key_rel_h, key_rel_w):
    global _NC_CACHE, LAST_RESULT
    x = np.asarray(x, np.float32)
    w_qkv = np.asarray(w_qkv, np.float32)
    w_conv = np.asarray(w_conv, np.float32)
    w_out = np.asarray(w_out, np.float32)
    key_rel_h = np.asarray(key_rel_h, np.float32)
    key_rel_w = np.asarray(key_rel_w, np.float32)

    # q/k proj weights: q rows get dkh^-0.5; both q and k scaled by 64 for
    # fp8 normal range (divided back out by the exp activation scale 2^-12)
    wq = w_qkv[:1024].copy() * 64.0
    wq[:DK] *= DKH ** -0.5
    wqt = wq.T                                    # (256 cin, 1024)
    wqk8 = _f8(wqt.reshape(2, 128, 1024).transpose(1, 0, 2).reshape(128, 2048))
    wvt = _bf(w_qkv[1024:].T)                     # (256, 256)
    wconvt = _bf(w_conv.transpose(1, 2, 3, 0).reshape(256, 9 * 256))
    wob = w_out.T * 64.0                           # (256 c, 256 o)
    wt2 = np.zeros((128, 2, 256), np.float32)
    for k in range(2):
        heads = [0, 1, 2, 3] if k == 0 else [7, 5, 6, 4]
        for g in range(4):
            rows = slice(32 * g, 32 * g + 32)
            wt2[rows, k, :] = wob[32 * heads[g]:32 * heads[g] + 32]
    woutt = _f8(wt2.reshape(128, 512))
    rel2 = np.concatenate([key_rel_w, key_rel_h], axis=1)  # (64, 126)
    relz = np.zeros((256, 126), np.float32)
    relz[0:64] = rel2         # parity 0: q rows at 0:64
    relz[192:256] = rel2      # parity 1: q rows at 64:128
    relz = _bf(relz)
    t = np.arange(L)
    ew = (t[None, :] // 32 == np.arange(32)[:, None]).astype(np.float32)
    eh = (t[None, :] % 32 == np.arange(32)[:, None]).astype(np.float32)
    e64 = np.concatenate([ew, eh], axis=0) * 64.0
    econst = _bf(np.concatenate([e64, e64], axis=0))       # (128, 1024)

    shared = dict(woutt=woutt, relz=relz, econst=econst)
    in_maps = []
    for b in range(B):
        xp = np.zeros((256, 34, 34), np.float32)
        xp[:, 1:33, 1:33] = x[b]
        xpA = _bf(xp.reshape(256, 1156))
        xb = x[b].reshape(256, 1024)
        xnA = _bf(xb)
        x8 = _f8(xb.reshape(2, 128, 1024).transpose(1, 0, 2).reshape(128, 2048))
        xw = np.concatenate([x8, wqk8], axis=1)            # (128, 4096) fp8
        bulka = np.concatenate(
            [xpA[0:128], wconvt[0:128], xpA[128:256], wconvt[128:256]], axis=1)
        bulkb = np.concatenate(
            [xnA[0:128], wvt[0:128], xnA[128:256], wvt[128:256]], axis=1)
        in_maps.append(dict(shared, bulka=np.ascontiguousarray(bulka),
                            bulkb=np.ascontiguousarray(bulkb),
                            xw8=np.ascontiguousarray(xw)))

    if _NC_CACHE is None:
        _NC_CACHE = build()
    res = run_bass_kernel_spmd(_NC_CACHE, in_maps, core_ids=list(range(8)),
                               trace=TRACE, **TRACE_KW)
    LAST_RESULT = res
    out = np.stack([res.results[i]["out"] for i in range(B)])
    return out.reshape(B, 512, H, W).astype(np.float32)
